# revision 4
# baseline (speedup 1.0000x reference)
"""Plan B kernel: proven primitives only.

8x single-index indirect gathers (as baseline, serialized on Pool SWDGE)
+ ONE XBAR dma_start_transpose (proven in bisect) replacing all PE
transposes + flipped matmuls (items on the moving free dim, K-major
output) + batched output DMAs.
"""

import numpy as np
import ml_dtypes

import concourse.bass as bass
import concourse.bacc as bacc
import concourse.mybir as mybir
from concourse.tile import TileContext
from concourse.bass_utils import run_bass_kernel_spmd

N_CORES = 8
L = 8192
LC = L // N_CORES
S = 256
K = 128
R = 4096
P = 128
NCH = LC // P     # 8 chunks of 128 items
NB = 2 * NCH      # 16 [128,128] blocks
NG = 2
GW = LC // NG


def build_bass() -> bass.Bass:
    nc = bacc.Bacc(trn_type="TRN2", dynamic_dma_scratch_size=131072)
    q = nc.declare_dram_parameter("q_bf16", [R, S], mybir.dt.bfloat16, isOutput=False)
    idx = nc.declare_dram_parameter("idx", [P, NCH], mybir.dt.int32, isOutput=False)
    emb = nc.declare_dram_parameter("embp", [P, S], mybir.dt.bfloat16, isOutput=False)
    out = nc.declare_dram_parameter("out", [K, LC], mybir.dt.float32, isOutput=True)

    with (
        TileContext(nc) as tc,
        tc.tile_pool(name="main", bufs=1) as pool,
        tc.tile_pool(name="acc", bufs=NG, space="PSUM") as apsum,
    ):
        idx_t = pool.tile([P, NCH], mybir.dt.int32)
        nc.sync.dma_start(out=idx_t[:], in_=idx[:])
        emb_t = pool.tile([P, S], mybir.dt.bfloat16)
        nc.scalar.dma_start(out=emb_t[:], in_=emb[:])

        # q_all[p, c*256+s] = Q[idx[p,c], s]; 8 single-idx gathers (HW-proven)
        q_all = pool.tile([P, NCH * S], mybir.dt.bfloat16)
        for c in range(NCH):
            nc.gpsimd.indirect_dma_start(
                out=q_all[:, c * S : (c + 1) * S],
                out_offset=None,
                in_=q[:],
                in_offset=bass.IndirectOffsetOnAxis(ap=idx_t[:, c : c + 1], axis=0),
            )

        # qT[s', 2c+e, i] = Q[items[c*128+i], e*128+s'] via XBAR (HW-proven),
        # split in halves so matmuls overlap the later gathers.
        qT = pool.tile([P, NB, P], mybir.dt.bfloat16)
        half = NCH * S // 2
        for h in range(2):
            nc.scalar.dma_start_transpose(
                out=qT[:, h * (NB // 2) : (h + 1) * (NB // 2), :],
                in_=q_all[:, h * half : (h + 1) * half],
            )

        qTr = qT[:].rearrange("s (c e) i -> s e c i", e=2)

        o_all = pool.tile([K, LC], mybir.dt.float32)
        for g in range(NG):
            ps = apsum.tile([K, GW], mybir.dt.float32, tag="ps")
            for e in range(2):
                nc.tensor.matmul(
                    ps[:],
                    emb_t[:, e * K : (e + 1) * K],
                    qTr[:, e, g * (NCH // NG) : (g + 1) * (NCH // NG), :],
                    start=(e == 0),
                    stop=(e == 1),
                )
            nc.vector.tensor_copy(o_all[:, g * GW : (g + 1) * GW], ps[:])
            eng = nc.sync if g == 0 else nc.scalar
            eng.dma_start(
                out=out[:, g * GW : (g + 1) * GW],
                in_=o_all[:, g * GW : (g + 1) * GW],
            )

    nc.compile()
    return nc


_CACHE: dict = {}


def get_nc() -> bass.Bass:
    if "nc" not in _CACHE:
        _CACHE["nc"] = build_bass()
    return _CACHE["nc"]


def make_in_maps(user, Q_matrix, items, skill_embedding):
    user = int(np.asarray(user))
    Q = np.asarray(Q_matrix, dtype=np.float32)
    items = np.asarray(items).astype(np.int64)
    E = np.ascontiguousarray(np.asarray(skill_embedding)[user], dtype=np.float32)
    q_bf = Q.astype(ml_dtypes.bfloat16)
    embp = np.ascontiguousarray(
        E.reshape(2, P, K).transpose(1, 0, 2).reshape(P, S).astype(ml_dtypes.bfloat16)
    )

    in_maps = []
    for i in range(N_CORES):
        it = items[i * LC : (i + 1) * LC].astype(np.int32)
        idx_arr = np.ascontiguousarray(it.reshape(NCH, P).T)
        in_maps.append({"q_bf16": q_bf, "idx": idx_arr, "embp": embp})
    return in_maps


def kernel(user, Q_matrix, items, skill_embedding, _trace=False, _result_box=None):
    in_maps = make_in_maps(user, Q_matrix, items, skill_embedding)
    res = run_bass_kernel_spmd(get_nc(), in_maps, list(range(N_CORES)), trace=_trace)
    if _result_box is not None:
        _result_box.append(res)
    out = np.concatenate(
        [np.asarray(res.results[i]["out"]).T for i in range(N_CORES)], axis=0
    )
    return np.ascontiguousarray(out, dtype=np.float32)
